# revision 3
# baseline (speedup 1.0000x reference)
"""Trainium2 Bass kernel for pointer-generator final-distribution (scatter_memory).

out[r, v] = p_gens[r] * vocab_ds[r, v]  (+ (1-p_gens[r])*attns[r, l_win]  at
v == sources[l, b(r)], duplicate source ids resolved last-occurrence-wins)

Strategy (8 NeuronCores, SPMD), bf16 streaming (DMA/HBM-bound):
  - The kernel is pure memory streaming: every element is read once and
    written once. Per-core HBM limit is ~358 GB/s, so bytes are the only
    lever. rel-err tolerance is 2e-2; bf16 (~0.4% err) halves traffic vs
    f32: host uploads vocab as bf16, device computes/stores bf16, host
    upconverts. ~52 MB/core -> ~145 us floor (f32 was ~103 MB -> ~290 us).
  - Shard by batch column: core k owns b in {4k..4k+3}, all T decoder steps
    (rows r = t*B + b). Host pre-gathers rows b-major so device DMAs are
    contiguous; two 128-row groups per core (2 b's x 64 t each).
  - Stream out = p * vocab through SBUF ([128, 8192] bf16 tiles = 2 MB
    DMAs, ACT does the per-partition scale). Loads on the sync HWDGE
    queue, stores on the scalar HWDGE queue — separate FIFOs avoid
    head-of-line blocking.
  - The scatter is applied in SBUF before the store via a compact one-hot
    matmul on the (otherwise idle) PE: for each 512-wide subtile, host
    bakes a [K, 128] block of bf16 update values (update k x row,
    block-diagonal over the two b's) and relative target columns ck;
    device builds the one-hot [K, 512] with is_equal(iota, ck) on DVE and
    PE computes proj = vals.T @ onehot into PSUM; DVE adds it into the
    streamed tile.
"""

import numpy as np

N_CORES = 8
WIN = 8192
SUB = 512


def _host_prep(vocab_ds, attns, p_gens, sources, T):
    import ml_dtypes
    f32 = np.float32
    bf16 = ml_dtypes.bfloat16
    vocab_ds = np.ascontiguousarray(vocab_ds, dtype=f32)
    attns = np.ascontiguousarray(attns, dtype=f32)
    p_gens = np.ascontiguousarray(p_gens, dtype=f32)
    src = np.asarray(sources).astype(np.int64)
    rows, V = vocab_ds.shape
    L, B = src.shape
    assert rows == T * B

    ag = (f32(1.0) - p_gens) * attns  # gated copy dist, [rows, L]

    # winners per batch column: duplicate source ids -> last occurrence wins
    wins = []
    for b in range(B):
        d = {}
        col = src[:, b]
        for l in range(L):
            d[int(col[l])] = l
        cols = np.fromiter(d.keys(), dtype=np.int64)
        ls = np.fromiter(d.values(), dtype=np.int64)
        o = np.argsort(cols)
        wins.append((cols[o], ls[o]))

    NW = (V + WIN - 1) // WIN
    # subtile geometry, shared by all cores/groups
    sub_geom = []  # (w, s, c0_abs, width)
    for w in range(NW):
        ww = min(WIN, V - w * WIN)
        for s in range((ww + SUB - 1) // SUB):
            sub_geom.append((w, s, w * WIN + s * SUB, min(SUB, ww - s * SUB)))
    NS = len(sub_geom)
    sub_of = {}
    for i, (w, s, c0, wd) in enumerate(sub_geom):
        sub_of[(w, s)] = i

    BPC = B // N_CORES  # 4
    G = BPC // 2        # 2 groups of 2 b's

    # bucket updates per (core, g, subtile)
    upd = [[[[] for _ in range(NS)] for _ in range(G)] for _ in range(N_CORES)]
    for core in range(N_CORES):
        for g in range(G):
            for half in range(2):
                b = core * BPC + g * 2 + half
                cols, ls = wins[b]
                for c, l in zip(cols.tolist(), ls.tolist()):
                    w = c // WIN
                    s = (c - w * WIN) // SUB
                    i = sub_of[(w, s)]
                    upd[core][g][i].append((half, c, l))

    # uniform-per-(g, subtile) K across cores
    K_ws = [[max(len(upd[core][g][i]) for core in range(N_CORES)) for i in range(NS)]
            for g in range(G)]
    KMAX = [max(K_ws[g]) if NS else 0 for g in range(G)]
    assert all(k <= 128 for g in range(G) for k in K_ws[g]), \
        "subtile update count exceeds the 128-partition budget"
    # per-(g, window): first subtile index, #subtiles, max K (partition
    # extent of the vals load for that window)
    win_info = []
    for g in range(G):
        wi = []
        for w in range(NW):
            idxs = [i for i, (w2, s2, _, _) in enumerate(sub_geom) if w2 == w]
            i0, nsub = idxs[0], len(idxs)
            kw = max(K_ws[g][i] for i in idxs)
            wi.append((i0, nsub, kw))
        win_info.append(wi)

    # per-core device inputs
    in_maps = []
    for core in range(N_CORES):
        m = {}
        for g in range(G):
            row_idx = []
            for half in range(2):
                b = core * BPC + g * 2 + half
                row_idx.extend(t * B + b for t in range(T))
            row_idx = np.asarray(row_idx)
            m[f"vocab{g}"] = vocab_ds[row_idx].astype(bf16)
            m[f"pgen{g}"] = p_gens[row_idx]
            vals = np.zeros((128, NS * 128), dtype=f32)
            ck = np.full((128, NS), -1.0, dtype=f32)
            for i in range(NS):
                w, s, c0, wd = sub_geom[i]
                for k, (half, c, l) in enumerate(upd[core][g][i]):
                    # rows of this b occupy partitions half*T .. half*T+T
                    r0 = half * T
                    vals[k, i * 128 + r0: i * 128 + r0 + T] = ag[row_idx[r0: r0 + T], l]
                    ck[k, i] = f32(c - c0)
            m[f"vals{g}"] = vals.astype(bf16)
            m[f"ck{g}"] = ck
        m["iota"] = np.broadcast_to(
            np.arange(SUB, dtype=f32), (128, SUB)).copy()
        in_maps.append(m)

    meta = dict(V=V, T=T, B=B, NW=NW, NS=NS, G=G, sub_geom=sub_geom,
                sub_of=sub_of, K_ws=K_ws, KMAX=KMAX, BPC=BPC,
                win_info=win_info)
    return in_maps, meta


def _build_nc(meta):
    from concourse import bacc, mybir

    V, NW, NS, G = meta["V"], meta["NW"], meta["NS"], meta["G"]
    sub_geom, K_ws, KMAX = meta["sub_geom"], meta["K_ws"], meta["KMAX"]
    f32 = mybir.dt.float32

    bf16 = mybir.dt.bfloat16
    nc = bacc.Bacc(None, target_bir_lowering=False, debug=False)
    vocab = [nc.declare_dram_parameter(f"vocab{g}", [128, V], bf16, isOutput=False)
             for g in range(G)]
    pgen = [nc.declare_dram_parameter(f"pgen{g}", [128, 1], f32, isOutput=False)
            for g in range(G)]
    vals = [nc.declare_dram_parameter(f"vals{g}", [128, NS * 128], bf16, isOutput=False)
            for g in range(G)]
    ck = [nc.declare_dram_parameter(f"ck{g}", [128, NS], f32, isOutput=False)
          for g in range(G)]
    iota = nc.declare_dram_parameter("iota", [128, SUB], f32, isOutput=False)
    out = [nc.declare_dram_parameter(f"out{g}", [128, V], bf16, isOutput=True)
           for g in range(G)]

    from concourse.tile import TileContext

    win_info = meta["win_info"]
    with TileContext(nc) as tc:
        with tc.tile_pool(name="io", bufs=6) as io_pool, \
             tc.tile_pool(name="small", bufs=1) as small, \
             tc.tile_pool(name="oh", bufs=8) as oh_pool, \
             tc.tile_pool(name="psum", bufs=8, space="PSUM") as psum_pool:

            # hoist all small preloads ahead of the streaming loop
            iota_t = small.tile([128, SUB], f32)
            nc.sync.dma_start(out=iota_t[:], in_=iota[:])
            p_t, ck_t, vals_t = [], [], []
            for g in range(G):
                p_t.append(small.tile([128, 1], f32, tag=f"p{g}", name=f"p{g}"))
                nc.sync.dma_start(out=p_t[g][:], in_=pgen[g][:])
                ck_t.append(small.tile([128, NS], f32, tag=f"ck{g}", name=f"ckt{g}"))
                nc.sync.dma_start(out=ck_t[g][:], in_=ck[g][:])
                kmax = max(wi[2] for wi in win_info[g])
                vals_t.append(small.tile([128, NS * 128], bf16, tag=f"vals{g}",
                                         name=f"valst{g}"))
                nc.sync.dma_start(out=vals_t[g][:kmax, :], in_=vals[g][:kmax, :])

            for g in range(G):
                for w in range(NW):
                    c0w = w * WIN
                    ww = min(WIN, V - c0w)
                    i0, nsub, kw = win_info[g][w]
                    t = io_pool.tile([128, WIN], bf16, tag="io")
                    nc.sync.dma_start(out=t[:, :ww], in_=vocab[g][:, c0w:c0w + ww])
                    nc.scalar.activation(
                        t[:, :ww], t[:, :ww],
                        mybir.ActivationFunctionType.Copy, scale=p_t[g][:, :1])
                    for s in range(nsub):
                        i = i0 + s
                        K = K_ws[g][i]
                        if K == 0:
                            continue
                        _, _, c0, wd = sub_geom[i]
                        oh = oh_pool.tile([128, SUB], bf16, tag="oh")
                        nc.vector.tensor_scalar(
                            out=oh[:K, :wd], in0=iota_t[:K, :wd],
                            scalar1=ck_t[g][:K, i:i + 1], scalar2=None,
                            op0=mybir.AluOpType.is_equal)
                        ps = psum_pool.tile([128, SUB], f32, tag="ps")
                        nc.tensor.matmul(
                            out=ps[:, :wd],
                            lhsT=vals_t[g][:K, i * 128:(i + 1) * 128],
                            rhs=oh[:K, :wd],
                            start=True, stop=True)
                        lo = c0 - c0w
                        nc.vector.tensor_add(
                            out=t[:, lo:lo + wd], in0=t[:, lo:lo + wd],
                            in1=ps[:, :wd])
                    nc.scalar.dma_start(out=out[g][:, c0w:c0w + ww], in_=t[:, :ww])
    nc.finalize()
    return nc


def kernel(vocab_ds, attns, p_gens, sources, decoder_batch_len):
    T = int(decoder_batch_len)
    in_maps, meta = _host_prep(vocab_ds, attns, p_gens, sources, T)
    nc = _build_nc(meta)

    from concourse.bass_utils import run_bass_kernel_spmd
    res = run_bass_kernel_spmd(nc, in_maps, list(range(N_CORES)))

    rows, V = np.asarray(vocab_ds).shape
    B, BPC, G = meta["B"], meta["BPC"], meta["G"]
    full = np.empty((rows, V), dtype=np.float32)
    for core in range(N_CORES):
        for g in range(G):
            blk = np.asarray(res.results[core][f"out{g}"], dtype=np.float32)
            for half in range(2):
                b = core * BPC + g * 2 + half
                full[b::B] = blk[half * T:(half + 1) * T]
    return full


# revision 5
# speedup vs baseline: 1.1814x; 1.1814x over previous
"""Trainium2 Bass kernel for pointer-generator final-distribution (scatter_memory).

out[r, v] = p_gens[r] * vocab_ds[r, v]  (+ (1-p_gens[r])*attns[r, l_win]  at
v == sources[l, b(r)], duplicate source ids resolved last-occurrence-wins)

Strategy (8 NeuronCores, SPMD), bf16 streaming (DMA/HBM-bound):
  - Pure memory streaming: every element is read once and written once.
    Per-core HBM limit is ~358 GB/s, so bytes are the only lever. rel-err
    tolerance is 2e-2; bf16 (~1% err) halves traffic vs f32: host uploads
    vocab as bf16, device computes/stores bf16, host upconverts.
  - Shard by batch column: core k owns b in {4k..4k+3}, all T decoder steps
    (rows r = t*B + b). Host pre-gathers rows b-major so device DMAs are
    contiguous; two 128-row groups per core (2 b's x 64 t each).
  - One engine, one job (no cross-engine semaphore stalls in a stream):
    SP(sync) issues all loads, ACT does the per-partition p-scale,
    PE matmuls host-baked one-hots into PSUM (scatter projection),
    DVE merges PSUM into the tile, GPSIMD issues stores via SWDGE.
  - Scatter: for each 1024-wide subtile, host bakes [K, 128] bf16 update
    values (block-diagonal over the two b's) and the one-hot [K, 1024]
    selector (is_equal was measured to cost 92us of DVE - host-baking it
    is ~6 MB of extra DMA instead). PE computes vals.T @ onehot -> PSUM;
    DVE adds 2048-wide PSUM chunks into the streamed tile (PSUM operands
    run DVE at 1x, so coalescing to 2048 amortizes instr overhead).
"""

import numpy as np

N_CORES = 8
WIN = 8192
SUB = 512
KCAP = 24


def _host_prep(vocab_ds, attns, p_gens, sources, T):
    import ml_dtypes
    f32 = np.float32
    bf16 = ml_dtypes.bfloat16
    vocab_ds = np.ascontiguousarray(vocab_ds, dtype=f32)
    attns = np.ascontiguousarray(attns, dtype=f32)
    p_gens = np.ascontiguousarray(p_gens, dtype=f32)
    src = np.asarray(sources).astype(np.int64)
    rows, V = vocab_ds.shape
    L, B = src.shape
    assert rows == T * B

    ag = (f32(1.0) - p_gens) * attns  # gated copy dist, [rows, L]

    # winners per batch column: duplicate source ids -> last occurrence wins
    wins = []
    for b in range(B):
        d = {}
        col = src[:, b]
        for l in range(L):
            d[int(col[l])] = l
        cols = np.fromiter(d.keys(), dtype=np.int64)
        ls = np.fromiter(d.values(), dtype=np.int64)
        o = np.argsort(cols)
        wins.append((cols[o], ls[o]))

    NW = (V + WIN - 1) // WIN
    # subtile geometry, shared by all cores/groups
    sub_geom = []  # (w, s, c0_abs, width)
    for w in range(NW):
        ww = min(WIN, V - w * WIN)
        for s in range((ww + SUB - 1) // SUB):
            sub_geom.append((w, s, w * WIN + s * SUB, min(SUB, ww - s * SUB)))
    NS = len(sub_geom)
    sub_of = {}
    for i, (w, s, c0, wd) in enumerate(sub_geom):
        sub_of[(w, s)] = i

    BPC = B // N_CORES  # 4
    G = BPC // 2        # 2 groups of 2 b's

    # bucket updates per (core, g, subtile)
    upd = [[[[] for _ in range(NS)] for _ in range(G)] for _ in range(N_CORES)]
    for core in range(N_CORES):
        for g in range(G):
            for half in range(2):
                b = core * BPC + g * 2 + half
                cols, ls = wins[b]
                for c, l in zip(cols.tolist(), ls.tolist()):
                    w = c // WIN
                    s = (c - w * WIN) // SUB
                    i = sub_of[(w, s)]
                    upd[core][g][i].append((half, c, l))

    # uniform-per-(g, subtile) K across cores; >=1 so every PSUM region is
    # written (a zero one-hot row yields zeros after start=True reset)
    K_ws = [[max(1, max(len(upd[core][g][i]) for core in range(N_CORES)))
             for i in range(NS)] for g in range(G)]
    assert all(k <= KCAP for g in range(G) for k in K_ws[g]), \
        "subtile update count exceeds KCAP"
    # per-(g, window): first subtile index, #subtiles, max K in window
    win_info = []
    for g in range(G):
        wi = []
        for w in range(NW):
            idxs = [i for i, (w2, s2, _, _) in enumerate(sub_geom) if w2 == w]
            i0, nsub = idxs[0], len(idxs)
            kw = max(K_ws[g][i] for i in idxs)
            wi.append((i0, nsub, kw))
        win_info.append(wi)

    # per-core device inputs
    in_maps = []
    for core in range(N_CORES):
        m = {}
        for g in range(G):
            row_idx = []
            for half in range(2):
                b = core * BPC + g * 2 + half
                row_idx.extend(t * B + b for t in range(T))
            row_idx = np.asarray(row_idx)
            m[f"vocab{g}"] = vocab_ds[row_idx].astype(bf16)
            m[f"pgen{g}"] = p_gens[row_idx]
            vals = np.zeros((KCAP, NS * 128), dtype=f32)
            oh = np.zeros((KCAP, NS * SUB), dtype=bf16)
            for i in range(NS):
                w, s, c0, wd = sub_geom[i]
                for k, (half, c, l) in enumerate(upd[core][g][i]):
                    # rows of this b occupy partitions half*T .. half*T+T
                    r0 = half * T
                    vals[k, i * 128 + r0: i * 128 + r0 + T] = ag[row_idx[r0: r0 + T], l]
                    oh[k, i * SUB + (c - c0)] = bf16(1.0)
            m[f"vals{g}"] = vals.astype(bf16)
            m[f"oh{g}"] = oh
        in_maps.append(m)

    meta = dict(V=V, T=T, B=B, NW=NW, NS=NS, G=G, sub_geom=sub_geom,
                sub_of=sub_of, K_ws=K_ws, BPC=BPC, win_info=win_info)
    return in_maps, meta


def _build_nc(meta):
    from concourse import bacc, mybir

    V, NW, NS, G = meta["V"], meta["NW"], meta["NS"], meta["G"]
    sub_geom, K_ws = meta["sub_geom"], meta["K_ws"]
    f32 = mybir.dt.float32

    bf16 = mybir.dt.bfloat16
    nc = bacc.Bacc(None, target_bir_lowering=False, debug=False)
    vocab = [nc.declare_dram_parameter(f"vocab{g}", [128, V], bf16, isOutput=False)
             for g in range(G)]
    pgen = [nc.declare_dram_parameter(f"pgen{g}", [128, 1], f32, isOutput=False)
            for g in range(G)]
    vals = [nc.declare_dram_parameter(f"vals{g}", [KCAP, NS * 128], bf16, isOutput=False)
            for g in range(G)]
    ohp = [nc.declare_dram_parameter(f"oh{g}", [KCAP, NS * SUB], bf16, isOutput=False)
           for g in range(G)]
    out = [nc.declare_dram_parameter(f"out{g}", [128, V], bf16, isOutput=True)
           for g in range(G)]

    from concourse.tile import TileContext

    win_info = meta["win_info"]
    with TileContext(nc) as tc:
        with tc.tile_pool(name="io", bufs=6) as io_pool, \
             tc.tile_pool(name="small", bufs=1) as small, \
             tc.tile_pool(name="oh", bufs=3) as oh_pool, \
             tc.tile_pool(name="psum", bufs=2, space="PSUM") as psum_pool:

            p_t, vals_t = [], []
            for g in range(G):
                p_t.append(small.tile([128, 1], f32, tag=f"p{g}", name=f"p{g}"))
                nc.sync.dma_start(out=p_t[g][:], in_=pgen[g][:])
                kmax = max(wi[2] for wi in win_info[g])
                vals_t.append(small.tile([KCAP, NS * 128], bf16, tag=f"vals{g}",
                                         name=f"valst{g}"))
                nc.sync.dma_start(out=vals_t[g][:kmax, :], in_=vals[g][:kmax, :])

            for g in range(G):
                for w in range(NW):
                    c0w = w * WIN
                    ww = min(WIN, V - c0w)
                    i0, nsub, kw = win_info[g][w]
                    oh_t = oh_pool.tile([KCAP, WIN], bf16, tag="oh", name="oht")
                    nc.sync.dma_start(
                        out=oh_t[:kw, :nsub * SUB],
                        in_=ohp[g][:kw, i0 * SUB:(i0 + nsub) * SUB])
                    t = io_pool.tile([128, WIN], bf16, tag="io")
                    nc.sync.dma_start(out=t[:, :ww], in_=vocab[g][:, c0w:c0w + ww])
                    nc.scalar.activation(
                        t[:, :ww], t[:, :ww],
                        mybir.ActivationFunctionType.Copy, scale=p_t[g][:, :1])
                    # PSUM chunks of 4 subtiles (2048 cols) -> one DVE merge each
                    for s0 in range(0, nsub, 4):
                        ns = min(4, nsub - s0)
                        ck_lo = s0 * SUB
                        ck_w = sum(sub_geom[i0 + s0 + j][3] for j in range(ns))
                        ps = psum_pool.tile([128, 4 * SUB], f32, tag="ps", name="ps")
                        for j in range(ns):
                            i = i0 + s0 + j
                            K = K_ws[g][i]
                            _, _, c0, wd = sub_geom[i]
                            nc.tensor.matmul(
                                out=ps[:, j * SUB:j * SUB + wd],
                                lhsT=vals_t[g][:K, i * 128:(i + 1) * 128],
                                rhs=oh_t[:K, (s0 + j) * SUB:(s0 + j) * SUB + wd],
                                start=True, stop=True)
                        nc.vector.tensor_add(
                            out=t[:, ck_lo:ck_lo + ck_w],
                            in0=t[:, ck_lo:ck_lo + ck_w],
                            in1=ps[:, :ck_w])
                    nc.gpsimd.dma_start(out=out[g][:, c0w:c0w + ww], in_=t[:, :ww])
    nc.finalize()
    return nc


def kernel(vocab_ds, attns, p_gens, sources, decoder_batch_len):
    T = int(decoder_batch_len)
    in_maps, meta = _host_prep(vocab_ds, attns, p_gens, sources, T)
    nc = _build_nc(meta)

    from concourse.bass_utils import run_bass_kernel_spmd
    res = run_bass_kernel_spmd(nc, in_maps, list(range(N_CORES)))

    rows, V = np.asarray(vocab_ds).shape
    B, BPC, G = meta["B"], meta["BPC"], meta["G"]
    full = np.empty((rows, V), dtype=np.float32)
    for core in range(N_CORES):
        for g in range(G):
            blk = np.asarray(res.results[core][f"out{g}"], dtype=np.float32)
            for half in range(2):
                b = core * BPC + g * 2 + half
                full[b::B] = blk[half * T:(half + 1) * T]
    return full


# revision 7
# speedup vs baseline: 1.2370x; 1.0470x over previous
"""Trainium2 Bass kernel for pointer-generator final-distribution (scatter_memory).

out[r, v] = p_gens[r] * vocab_ds[r, v]  (+ (1-p_gens[r])*attns[r, l_win]  at
v == sources[l, b(r)], duplicate source ids resolved last-occurrence-wins)

Strategy (8 NeuronCores, SPMD), bf16 streaming (DMA/HBM-bound):
  - Pure memory streaming: every element is read once and written once.
    Per-core HBM limit is ~358 GB/s, so bytes are the only lever. rel-err
    tolerance is 2e-2; bf16 (~1% err) halves traffic vs f32: host uploads
    vocab as bf16, device computes/stores bf16, host upconverts.
  - Shard by batch column: core k owns b in {4k..4k+3}, all T decoder steps
    (rows r = t*B + b). Host pre-gathers rows b-major so device DMAs are
    contiguous; two 128-row groups per core (2 b's x 64 t each).
  - One engine, one job (no cross-engine semaphore stalls in a stream):
    SP(sync) issues all loads, ACT does the per-partition p-scale,
    PE matmuls host-baked one-hots into PSUM (scatter projection),
    DVE merges PSUM into the tile, GPSIMD issues stores via SWDGE.
  - Scatter: for each 1024-wide subtile, host bakes [K, 128] bf16 update
    values (block-diagonal over the two b's) and the one-hot [K, 1024]
    selector (is_equal was measured to cost 92us of DVE - host-baking it
    is ~6 MB of extra DMA instead). PE computes vals.T @ onehot -> PSUM;
    DVE adds 2048-wide PSUM chunks into the streamed tile (PSUM operands
    run DVE at 1x, so coalescing to 2048 amortizes instr overhead).
"""

import numpy as np

N_CORES = 8
WIN = 8192
SUB = 512
KCAP = 24


def _host_prep(vocab_ds, attns, p_gens, sources, T):
    import ml_dtypes
    f32 = np.float32
    bf16 = ml_dtypes.bfloat16
    vocab_ds = np.ascontiguousarray(vocab_ds, dtype=f32)
    attns = np.ascontiguousarray(attns, dtype=f32)
    p_gens = np.ascontiguousarray(p_gens, dtype=f32)
    src = np.asarray(sources).astype(np.int64)
    rows, V = vocab_ds.shape
    L, B = src.shape
    assert rows == T * B

    ag = (f32(1.0) - p_gens) * attns  # gated copy dist, [rows, L]

    # winners per batch column: duplicate source ids -> last occurrence wins
    wins = []
    for b in range(B):
        d = {}
        col = src[:, b]
        for l in range(L):
            d[int(col[l])] = l
        cols = np.fromiter(d.keys(), dtype=np.int64)
        ls = np.fromiter(d.values(), dtype=np.int64)
        o = np.argsort(cols)
        wins.append((cols[o], ls[o]))

    NW = (V + WIN - 1) // WIN
    # subtile geometry, shared by all cores/groups
    sub_geom = []  # (w, s, c0_abs, width)
    for w in range(NW):
        ww = min(WIN, V - w * WIN)
        for s in range((ww + SUB - 1) // SUB):
            sub_geom.append((w, s, w * WIN + s * SUB, min(SUB, ww - s * SUB)))
    NS = len(sub_geom)
    sub_of = {}
    for i, (w, s, c0, wd) in enumerate(sub_geom):
        sub_of[(w, s)] = i

    BPC = B // N_CORES  # 4
    G = BPC // 2        # 2 groups of 2 b's

    # bucket updates per (core, g, subtile)
    upd = [[[[] for _ in range(NS)] for _ in range(G)] for _ in range(N_CORES)]
    for core in range(N_CORES):
        for g in range(G):
            for half in range(2):
                b = core * BPC + g * 2 + half
                cols, ls = wins[b]
                for c, l in zip(cols.tolist(), ls.tolist()):
                    w = c // WIN
                    s = (c - w * WIN) // SUB
                    i = sub_of[(w, s)]
                    upd[core][g][i].append((half, c, l))

    # uniform-per-(g, subtile) K across cores; >=1 so every PSUM region is
    # written (a zero one-hot row yields zeros after start=True reset)
    K_ws = [[max(1, max(len(upd[core][g][i]) for core in range(N_CORES)))
             for i in range(NS)] for g in range(G)]
    assert all(k <= KCAP for g in range(G) for k in K_ws[g]), \
        "subtile update count exceeds KCAP"
    # per-(g, window): first subtile index, #subtiles, max K in window
    win_info = []
    for g in range(G):
        wi = []
        for w in range(NW):
            idxs = [i for i, (w2, s2, _, _) in enumerate(sub_geom) if w2 == w]
            i0, nsub = idxs[0], len(idxs)
            kw = max(K_ws[g][i] for i in idxs)
            wi.append((i0, nsub, kw))
        win_info.append(wi)

    # per-core device inputs
    in_maps = []
    for core in range(N_CORES):
        m = {}
        for g in range(G):
            row_idx = []
            for half in range(2):
                b = core * BPC + g * 2 + half
                row_idx.extend(t * B + b for t in range(T))
            row_idx = np.asarray(row_idx)
            m[f"vocab{g}"] = vocab_ds[row_idx].astype(bf16)
            m[f"pgen{g}"] = p_gens[row_idx]
            vals = np.zeros((KCAP, NS * 128), dtype=f32)
            oh = np.zeros((KCAP, NS * SUB), dtype=bf16)
            for i in range(NS):
                w, s, c0, wd = sub_geom[i]
                for k, (half, c, l) in enumerate(upd[core][g][i]):
                    # rows of this b occupy partitions half*T .. half*T+T
                    r0 = half * T
                    vals[k, i * 128 + r0: i * 128 + r0 + T] = ag[row_idx[r0: r0 + T], l]
                    oh[k, i * SUB + (c - c0)] = bf16(1.0)
            m[f"vals{g}"] = vals.astype(bf16)
            m[f"oh{g}"] = oh
        in_maps.append(m)

    meta = dict(V=V, T=T, B=B, NW=NW, NS=NS, G=G, sub_geom=sub_geom,
                sub_of=sub_of, K_ws=K_ws, BPC=BPC, win_info=win_info)
    return in_maps, meta


def _build_nc(meta):
    from concourse import bacc, mybir

    V, NW, NS, G = meta["V"], meta["NW"], meta["NS"], meta["G"]
    sub_geom, K_ws = meta["sub_geom"], meta["K_ws"]
    f32 = mybir.dt.float32

    bf16 = mybir.dt.bfloat16
    nc = bacc.Bacc(None, target_bir_lowering=False, debug=False)
    vocab = [nc.declare_dram_parameter(f"vocab{g}", [128, V], bf16, isOutput=False)
             for g in range(G)]
    pgen = [nc.declare_dram_parameter(f"pgen{g}", [128, 1], f32, isOutput=False)
            for g in range(G)]
    vals = [nc.declare_dram_parameter(f"vals{g}", [KCAP, NS * 128], bf16, isOutput=False)
            for g in range(G)]
    ohp = [nc.declare_dram_parameter(f"oh{g}", [KCAP, NS * SUB], bf16, isOutput=False)
           for g in range(G)]
    out = [nc.declare_dram_parameter(f"out{g}", [128, V], bf16, isOutput=True)
           for g in range(G)]

    from concourse.tile import TileContext

    win_info = meta["win_info"]
    with TileContext(nc) as tc:
        with tc.tile_pool(name="io", bufs=6) as io_pool, \
             tc.tile_pool(name="small", bufs=1) as small, \
             tc.tile_pool(name="oh", bufs=3) as oh_pool, \
             tc.tile_pool(name="psum", bufs=2, space="PSUM") as psum_pool:

            p_t, vals_t = [], []
            for g in range(G):
                p_t.append(small.tile([128, 1], f32, tag=f"p{g}", name=f"p{g}"))
                nc.sync.dma_start(out=p_t[g][:], in_=pgen[g][:])
                kmax = max(wi[2] for wi in win_info[g])
                vals_t.append(small.tile([KCAP, NS * 128], bf16, tag=f"vals{g}",
                                         name=f"valst{g}"))
                nc.sync.dma_start(out=vals_t[g][:kmax, :], in_=vals[g][:kmax, :])

            for g in range(G):
                for w in range(NW):
                    c0w = w * WIN
                    ww = min(WIN, V - c0w)
                    i0, nsub, kw = win_info[g][w]
                    oh_t = oh_pool.tile([KCAP, WIN], bf16, tag="oh", name="oht")
                    nc.sync.dma_start(
                        out=oh_t[:kw, :nsub * SUB],
                        in_=ohp[g][:kw, i0 * SUB:(i0 + nsub) * SUB])
                    t = io_pool.tile([128, WIN], bf16, tag="io")
                    nc.sync.dma_start(out=t[:, :ww], in_=vocab[g][:, c0w:c0w + ww])
                    # PSUM chunks of 4 subtiles (2048 cols); one fused DVE
                    # pass per chunk does scale+merge: t = t*p + ps
                    for s0 in range(0, nsub, 4):
                        ns = min(4, nsub - s0)
                        ck_lo = s0 * SUB
                        ck_w = sum(sub_geom[i0 + s0 + j][3] for j in range(ns))
                        ps = psum_pool.tile([128, 4 * SUB], f32, tag="ps", name="ps")
                        for j in range(ns):
                            i = i0 + s0 + j
                            K = K_ws[g][i]
                            _, _, c0, wd = sub_geom[i]
                            nc.tensor.matmul(
                                out=ps[:, j * SUB:j * SUB + wd],
                                lhsT=vals_t[g][:K, i * 128:(i + 1) * 128],
                                rhs=oh_t[:K, (s0 + j) * SUB:(s0 + j) * SUB + wd],
                                start=True, stop=True)
                        nc.vector.scalar_tensor_tensor(
                            out=t[:, ck_lo:ck_lo + ck_w],
                            in0=t[:, ck_lo:ck_lo + ck_w],
                            scalar=p_t[g][:, :1],
                            in1=ps[:, :ck_w],
                            op0=mybir.AluOpType.mult,
                            op1=mybir.AluOpType.add)
                    nc.gpsimd.dma_start(out=out[g][:, c0w:c0w + ww], in_=t[:, :ww])
    nc.finalize()
    return nc


def kernel(vocab_ds, attns, p_gens, sources, decoder_batch_len):
    T = int(decoder_batch_len)
    in_maps, meta = _host_prep(vocab_ds, attns, p_gens, sources, T)
    nc = _build_nc(meta)

    from concourse.bass_utils import run_bass_kernel_spmd
    res = run_bass_kernel_spmd(nc, in_maps, list(range(N_CORES)))

    rows, V = np.asarray(vocab_ds).shape
    B, BPC, G = meta["B"], meta["BPC"], meta["G"]
    full = np.empty((rows, V), dtype=np.float32)
    for core in range(N_CORES):
        for g in range(G):
            blk = np.asarray(res.results[core][f"out{g}"], dtype=np.float32)
            for half in range(2):
                b = core * BPC + g * 2 + half
                full[b::B] = blk[half * T:(half + 1) * T]
    return full


# revision 9
# speedup vs baseline: 1.3335x; 1.0781x over previous
"""Trainium2 Bass kernel for pointer-generator final-distribution (scatter_memory).

out[r, v] = p_gens[r] * vocab_ds[r, v]  (+ (1-p_gens[r])*attns[r, l_win]  at
v == sources[l, b(r)], duplicate source ids resolved last-occurrence-wins)

Strategy (8 NeuronCores, SPMD), bf16 streaming (DMA/HBM-bound):
  - Pure memory streaming: every element is read once and written once.
    Per-core HBM limit is ~358 GB/s, so bytes are the only lever. rel-err
    tolerance is 2e-2; bf16 (~1% err) halves traffic vs f32: host uploads
    vocab as bf16, device computes/stores bf16, host upconverts.
  - Shard by batch column: core k owns b in {4k..4k+3}, all T decoder steps
    (rows r = t*B + b). Host pre-gathers rows b-major so device DMAs are
    contiguous; two 128-row groups per core (2 b's x 64 t each).
  - One engine, one job (no cross-engine semaphore stalls in a stream):
    SP(sync) issues all loads, ACT does the per-partition p-scale,
    PE matmuls host-baked one-hots into PSUM (scatter projection),
    DVE merges PSUM into the tile, GPSIMD issues stores via SWDGE.
  - Scatter: for each 1024-wide subtile, host bakes [K, 128] bf16 update
    values (block-diagonal over the two b's) and the one-hot [K, 1024]
    selector (is_equal was measured to cost 92us of DVE - host-baking it
    is ~6 MB of extra DMA instead). PE computes vals.T @ onehot -> PSUM;
    DVE adds 2048-wide PSUM chunks into the streamed tile (PSUM operands
    run DVE at 1x, so coalescing to 2048 amortizes instr overhead).
"""

import numpy as np

N_CORES = 8
WIN = 8192
SUB = 512
KCAP = 24


def _host_prep(vocab_ds, attns, p_gens, sources, T):
    import ml_dtypes
    f32 = np.float32
    bf16 = ml_dtypes.bfloat16
    vocab_ds = np.ascontiguousarray(vocab_ds, dtype=f32)
    attns = np.ascontiguousarray(attns, dtype=f32)
    p_gens = np.ascontiguousarray(p_gens, dtype=f32)
    src = np.asarray(sources).astype(np.int64)
    rows, V = vocab_ds.shape
    L, B = src.shape
    assert rows == T * B

    ag = (f32(1.0) - p_gens) * attns  # gated copy dist, [rows, L]

    # winners per batch column: duplicate source ids -> last occurrence wins
    wins = []
    for b in range(B):
        d = {}
        col = src[:, b]
        for l in range(L):
            d[int(col[l])] = l
        cols = np.fromiter(d.keys(), dtype=np.int64)
        ls = np.fromiter(d.values(), dtype=np.int64)
        o = np.argsort(cols)
        wins.append((cols[o], ls[o]))

    NW = (V + WIN - 1) // WIN
    # subtile geometry, shared by all cores/groups
    sub_geom = []  # (w, s, c0_abs, width)
    for w in range(NW):
        ww = min(WIN, V - w * WIN)
        for s in range((ww + SUB - 1) // SUB):
            sub_geom.append((w, s, w * WIN + s * SUB, min(SUB, ww - s * SUB)))
    NS = len(sub_geom)
    sub_of = {}
    for i, (w, s, c0, wd) in enumerate(sub_geom):
        sub_of[(w, s)] = i

    BPC = B // N_CORES  # 4
    G = BPC // 2        # 2 groups of 2 b's

    # bucket updates per (core, g, subtile)
    upd = [[[[] for _ in range(NS)] for _ in range(G)] for _ in range(N_CORES)]
    for core in range(N_CORES):
        for g in range(G):
            for half in range(2):
                b = core * BPC + g * 2 + half
                cols, ls = wins[b]
                for c, l in zip(cols.tolist(), ls.tolist()):
                    w = c // WIN
                    s = (c - w * WIN) // SUB
                    i = sub_of[(w, s)]
                    upd[core][g][i].append((half, c, l))

    # uniform-per-(g, subtile) K across cores; >=1 so every PSUM region is
    # written (a zero one-hot row yields zeros after start=True reset)
    K_ws = [[max(1, max(len(upd[core][g][i]) for core in range(N_CORES)))
             for i in range(NS)] for g in range(G)]
    assert all(k <= KCAP for g in range(G) for k in K_ws[g]), \
        "subtile update count exceeds KCAP"
    # per-(g, window): first subtile index, #subtiles, max K in window
    win_info = []
    for g in range(G):
        wi = []
        for w in range(NW):
            idxs = [i for i, (w2, s2, _, _) in enumerate(sub_geom) if w2 == w]
            i0, nsub = idxs[0], len(idxs)
            kw = max(K_ws[g][i] for i in idxs)
            wi.append((i0, nsub, kw))
        win_info.append(wi)

    # per-core device inputs
    in_maps = []
    for core in range(N_CORES):
        m = {}
        for g in range(G):
            row_idx = []
            for half in range(2):
                b = core * BPC + g * 2 + half
                row_idx.extend(t * B + b for t in range(T))
            row_idx = np.asarray(row_idx)
            m[f"vocab{g}"] = vocab_ds[row_idx].astype(bf16)
            m[f"pgen{g}"] = p_gens[row_idx]
            # merged per-subtile [K, 128 vals | 512 one-hot] blocks so the
            # scatter operands ship as one rotating-base DMA stream
            W = 128 + SUB
            ohv = np.zeros((KCAP, NS * W), dtype=f32)
            for i in range(NS):
                w, s, c0, wd = sub_geom[i]
                for k, (half, c, l) in enumerate(upd[core][g][i]):
                    # rows of this b occupy partitions half*T .. half*T+T
                    r0 = half * T
                    ohv[k, i * W + r0: i * W + r0 + T] = ag[row_idx[r0: r0 + T], l]
                    ohv[k, i * W + 128 + (c - c0)] = 1.0
            m[f"ohv{g}"] = ohv.astype(bf16)
        in_maps.append(m)

    meta = dict(V=V, T=T, B=B, NW=NW, NS=NS, G=G, sub_geom=sub_geom,
                sub_of=sub_of, K_ws=K_ws, BPC=BPC, win_info=win_info)
    return in_maps, meta


def _build_nc(meta):
    from concourse import bacc, mybir

    V, NW, NS, G = meta["V"], meta["NW"], meta["NS"], meta["G"]
    sub_geom, K_ws = meta["sub_geom"], meta["K_ws"]
    f32 = mybir.dt.float32

    bf16 = mybir.dt.bfloat16
    nc = bacc.Bacc(None, target_bir_lowering=False, debug=False)
    vocab = [nc.declare_dram_parameter(f"vocab{g}", [128, V], bf16, isOutput=False)
             for g in range(G)]
    pgen = [nc.declare_dram_parameter(f"pgen{g}", [128, 1], f32, isOutput=False)
            for g in range(G)]
    SW = 128 + SUB
    ohv = [nc.declare_dram_parameter(f"ohv{g}", [KCAP, NS * SW], bf16, isOutput=False)
           for g in range(G)]
    out = [nc.declare_dram_parameter(f"out{g}", [128, V], bf16, isOutput=True)
           for g in range(G)]

    from concourse.tile import TileContext

    win_info = meta["win_info"]
    # SBUF base partition alternates {0,64} per window so the narrow
    # (≤KCAP-partition) ohv DMAs spread over both SDMA engine halves
    # (matmul requires base partition in {0,32,64}, lhsT/rhs bases equal)
    SW = 128 + SUB
    with TileContext(nc) as tc:
        with tc.tile_pool(name="io", bufs=7) as io_pool, \
             tc.tile_pool(name="small", bufs=1) as small, \
             tc.tile_pool(name="oh", bufs=3) as oh_pool, \
             tc.tile_pool(name="psum", bufs=2, space="PSUM") as psum_pool:

            p_t = []
            for g in range(G):
                p_t.append(small.tile([128, 1], f32, tag=f"p{g}", name=f"p{g}"))

            for g in range(G):
                nc.sync.dma_start(out=p_t[g][:], in_=pgen[g][:])
                for w in range(NW):
                    c0w = w * WIN
                    ww = min(WIN, V - c0w)
                    i0, nsub, kw = win_info[g][w]
                    oo = 64 * ((w + g) % 2)
                    oh_t = oh_pool.tile([128, (WIN // SUB) * SW], bf16,
                                        tag="oh", name="oht")
                    nc.sync.dma_start(
                        out=oh_t[oo:oo + kw, :nsub * SW],
                        in_=ohv[g][:kw, i0 * SW:(i0 + nsub) * SW])
                    t = io_pool.tile([128, WIN], bf16, tag="io")
                    nc.sync.dma_start(out=t[:, :ww], in_=vocab[g][:, c0w:c0w + ww])
                    # PSUM chunks of 4 subtiles (2048 cols); one fused DVE
                    # pass per chunk does scale+merge: t = t*p + ps
                    for s0 in range(0, nsub, 4):
                        ns = min(4, nsub - s0)
                        ck_lo = s0 * SUB
                        ck_w = sum(sub_geom[i0 + s0 + j][3] for j in range(ns))
                        ps = psum_pool.tile([128, 4 * SUB], f32, tag="ps", name="ps")
                        for j in range(ns):
                            i = i0 + s0 + j
                            K = K_ws[g][i]
                            _, _, c0, wd = sub_geom[i]
                            sl = (s0 + j) * SW
                            nc.tensor.matmul(
                                out=ps[:, j * SUB:j * SUB + wd],
                                lhsT=oh_t[oo:oo + K, sl:sl + 128],
                                rhs=oh_t[oo:oo + K, sl + 128:sl + 128 + wd],
                                start=True, stop=True)
                        nc.vector.scalar_tensor_tensor(
                            out=t[:, ck_lo:ck_lo + ck_w],
                            in0=t[:, ck_lo:ck_lo + ck_w],
                            scalar=p_t[g][:, :1],
                            in1=ps[:, :ck_w],
                            op0=mybir.AluOpType.mult,
                            op1=mybir.AluOpType.add)
                    nc.scalar.dma_start(out=out[g][:, c0w:c0w + ww], in_=t[:, :ww])
    nc.finalize()
    return nc


def kernel(vocab_ds, attns, p_gens, sources, decoder_batch_len):
    T = int(decoder_batch_len)
    in_maps, meta = _host_prep(vocab_ds, attns, p_gens, sources, T)
    nc = _build_nc(meta)

    from concourse.bass_utils import run_bass_kernel_spmd
    res = run_bass_kernel_spmd(nc, in_maps, list(range(N_CORES)))

    rows, V = np.asarray(vocab_ds).shape
    B, BPC, G = meta["B"], meta["BPC"], meta["G"]
    full = np.empty((rows, V), dtype=np.float32)
    for core in range(N_CORES):
        for g in range(G):
            blk = np.asarray(res.results[core][f"out{g}"], dtype=np.float32)
            for half in range(2):
                b = core * BPC + g * 2 + half
                full[b::B] = blk[half * T:(half + 1) * T]
    return full


# revision 11
# speedup vs baseline: 1.4874x; 1.1154x over previous
"""Trainium2 Bass kernel for pointer-generator final-distribution (scatter_memory).

out[r, v] = p_gens[r] * vocab_ds[r, v]  (+ (1-p_gens[r])*attns[r, l_win]  at
v == sources[l, b(r)], duplicate source ids resolved last-occurrence-wins)

Strategy (8 NeuronCores, SPMD), bf16 streaming (DMA/HBM-bound):
  - Pure memory streaming: every element is read once and written once.
    Per-core HBM limit is ~358 GB/s, so bytes are the only lever. rel-err
    tolerance is 2e-2; bf16 (~1% err) halves traffic vs f32: host uploads
    vocab as bf16, device computes/stores bf16, host upconverts.
  - Shard by batch column: core k owns b in {4k..4k+3}, all T decoder steps
    (rows r = t*B + b). Host pre-gathers rows b-major so device DMAs are
    contiguous; two 128-row groups per core (2 b's x 64 t each).
  - One engine, one job (no cross-engine semaphore stalls in a stream):
    SP(sync) issues all loads, ACT does the per-partition p-scale,
    PE matmuls host-baked one-hots into PSUM (scatter projection),
    DVE merges PSUM into the tile, GPSIMD issues stores via SWDGE.
  - Scatter: for each 1024-wide subtile, host bakes [K, 128] bf16 update
    values (block-diagonal over the two b's) and the one-hot [K, 1024]
    selector (is_equal was measured to cost 92us of DVE - host-baking it
    is ~6 MB of extra DMA instead). PE computes vals.T @ onehot -> PSUM;
    DVE adds 2048-wide PSUM chunks into the streamed tile (PSUM operands
    run DVE at 1x, so coalescing to 2048 amortizes instr overhead).
"""

import numpy as np

N_CORES = 8
WIN = 8192
SUB = 512
KROW = 16


def _host_prep(vocab_ds, attns, p_gens, sources, T):
    import ml_dtypes
    f32 = np.float32
    bf16 = ml_dtypes.bfloat16
    vocab_ds = np.ascontiguousarray(vocab_ds, dtype=f32)
    attns = np.ascontiguousarray(attns, dtype=f32)
    p_gens = np.ascontiguousarray(p_gens, dtype=f32)
    src = np.asarray(sources).astype(np.int64)
    rows, V = vocab_ds.shape
    L, B = src.shape
    assert rows == T * B

    ag = (f32(1.0) - p_gens) * attns  # gated copy dist, [rows, L]

    # winners per batch column: duplicate source ids -> last occurrence wins
    wins = []
    for b in range(B):
        d = {}
        col = src[:, b]
        for l in range(L):
            d[int(col[l])] = l
        cols = np.fromiter(d.keys(), dtype=np.int64)
        ls = np.fromiter(d.values(), dtype=np.int64)
        o = np.argsort(cols)
        wins.append((cols[o], ls[o]))

    NW = (V + WIN - 1) // WIN
    # subtile geometry, shared by all cores/groups
    sub_geom = []  # (w, s, c0_abs, width)
    for w in range(NW):
        ww = min(WIN, V - w * WIN)
        for s in range((ww + SUB - 1) // SUB):
            sub_geom.append((w, s, w * WIN + s * SUB, min(SUB, ww - s * SUB)))
    NS = len(sub_geom)
    sub_of = {}
    for i, (w, s, c0, wd) in enumerate(sub_geom):
        sub_of[(w, s)] = i

    BPC = B // N_CORES  # 4
    G = BPC // 2        # 2 groups of 2 b's

    # bucket updates per (core, g, subtile)
    upd = [[[[] for _ in range(NS)] for _ in range(G)] for _ in range(N_CORES)]
    for core in range(N_CORES):
        for g in range(G):
            for half in range(2):
                b = core * BPC + g * 2 + half
                cols, ls = wins[b]
                for c, l in zip(cols.tolist(), ls.tolist()):
                    w = c // WIN
                    s = (c - w * WIN) // SUB
                    i = sub_of[(w, s)]
                    upd[core][g][i].append((half, c, l))

    # uniform-per-(g, subtile) K across cores; >=1 so every PSUM region is
    # written (a zero one-hot row yields zeros after start=True reset)
    K_ws = [[max(1, max(len(upd[core][g][i]) for core in range(N_CORES)))
             for i in range(NS)] for g in range(G)]
    assert all(k <= 2 * KROW for g in range(G) for k in K_ws[g]), \
        "subtile update count exceeds 2*KROW"
    # Every scatter job is a [KROW, 128+SUB] block (vals | one-hot); rows
    # beyond the real K are zero (numerically exact). Subtiles with
    # K > KROW get a second accumulate job (start=False). Exactly KROW
    # rows per DMA keeps the row->SDMA-engine round-robin balanced.
    # jobs[g][w] = list of (jidx, s_local, wd, start, stop, i, klo)
    jobs = [[[] for _ in range(NW)] for _ in range(G)]
    win_info = []  # per (g,w): (i0, nsub, j0, njobs)
    NJ = []
    for g in range(G):
        wi = []
        jidx = 0
        for w in range(NW):
            idxs = [i for i, (w2, s2, _, _) in enumerate(sub_geom) if w2 == w]
            i0, nsub = idxs[0], len(idxs)
            j0 = jidx
            for s, i in enumerate(idxs):
                K = K_ws[g][i]
                wd = sub_geom[i][3]
                if K <= KROW:
                    jobs[g][w].append((jidx, s, wd, True, True, i, 0))
                    jidx += 1
                else:
                    jobs[g][w].append((jidx, s, wd, True, False, i, 0))
                    jobs[g][w].append((jidx + 1, s, wd, False, True, i, KROW))
                    jidx += 2
            wi.append((i0, nsub, j0, jidx - j0))
        win_info.append(wi)
        NJ.append(jidx)

    # per-core device inputs
    in_maps = []
    for core in range(N_CORES):
        m = {}
        for g in range(G):
            row_idx = []
            for half in range(2):
                b = core * BPC + g * 2 + half
                row_idx.extend(t * B + b for t in range(T))
            row_idx = np.asarray(row_idx)
            m[f"vocab{g}"] = vocab_ds[row_idx].astype(bf16)
            m[f"pgen{g}"] = p_gens[row_idx]
            # merged per-job [KROW, 128 vals | 512 one-hot] blocks so the
            # scatter operands ship as one balanced DMA stream
            W = 128 + SUB
            ohv = np.zeros((KROW, NJ[g] * W), dtype=f32)
            for w in range(NW):
                for (jj, s, wd, st, sp, i, klo) in jobs[g][w]:
                    c0 = sub_geom[i][2]
                    ups = upd[core][g][i][klo:klo + KROW]
                    for k, (half, c, l) in enumerate(ups):
                        r0 = half * T
                        ohv[k, jj * W + r0: jj * W + r0 + T] = \
                            ag[row_idx[r0: r0 + T], l]
                        ohv[k, jj * W + 128 + (c - c0)] = 1.0
            m[f"ohv{g}"] = ohv.astype(bf16)
        in_maps.append(m)

    meta = dict(V=V, T=T, B=B, NW=NW, NS=NS, G=G, sub_geom=sub_geom,
                sub_of=sub_of, K_ws=K_ws, BPC=BPC, win_info=win_info,
                jobs=jobs, NJ=NJ)
    return in_maps, meta


def _build_nc(meta):
    from concourse import bacc, mybir

    V, NW, NS, G = meta["V"], meta["NW"], meta["NS"], meta["G"]
    sub_geom, K_ws = meta["sub_geom"], meta["K_ws"]
    f32 = mybir.dt.float32

    bf16 = mybir.dt.bfloat16
    nc = bacc.Bacc(None, target_bir_lowering=False, debug=False)
    vocab = [nc.declare_dram_parameter(f"vocab{g}", [128, V], bf16, isOutput=False)
             for g in range(G)]
    pgen = [nc.declare_dram_parameter(f"pgen{g}", [128, 1], f32, isOutput=False)
            for g in range(G)]
    SW = 128 + SUB
    NJ = meta["NJ"]
    ohv = [nc.declare_dram_parameter(f"ohv{g}", [KROW, NJ[g] * SW], bf16, isOutput=False)
           for g in range(G)]
    out = [nc.declare_dram_parameter(f"out{g}", [128, V], bf16, isOutput=True)
           for g in range(G)]

    from concourse.tile import TileContext

    win_info = meta["win_info"]
    jobs = meta["jobs"]
    # SBUF base partition alternates {0,64} per window so the narrow
    # (KROW-partition) ohv DMAs spread over both SDMA engine halves
    # (matmul requires base partition in {0,32,64}, lhsT/rhs bases equal)
    SW = 128 + SUB
    with TileContext(nc) as tc:
        with tc.tile_pool(name="io", bufs=7) as io_pool, \
             tc.tile_pool(name="small", bufs=1) as small, \
             tc.tile_pool(name="oh", bufs=3) as oh_pool, \
             tc.tile_pool(name="psum", bufs=2, space="PSUM") as psum_pool:

            p_t = []
            for g in range(G):
                p_t.append(small.tile([128, 1], f32, tag=f"p{g}", name=f"p{g}"))

            for g in range(G):
                nc.sync.dma_start(out=p_t[g][:], in_=pgen[g][:])
                for w in range(NW):
                    c0w = w * WIN
                    ww = min(WIN, V - c0w)
                    i0, nsub, j0, njobs = win_info[g][w]
                    oo = 64 * ((w + g) % 2)
                    t = io_pool.tile([128, WIN], bf16, tag="io")
                    nc.sync.dma_start(out=t[:, :ww], in_=vocab[g][:, c0w:c0w + ww])
                    oh_t = oh_pool.tile([128, 18 * SW], bf16,
                                        tag="oh", name="oht")
                    nc.sync.dma_start(
                        out=oh_t[oo:oo + KROW, :njobs * SW],
                        in_=ohv[g][:, j0 * SW:(j0 + njobs) * SW])
                    # PSUM chunks of 4 subtiles (2048 cols); one fused DVE
                    # pass per chunk does scale+merge: t = t*p + ps
                    wjobs = jobs[g][w]
                    for s0 in range(0, nsub, 4):
                        ns = min(4, nsub - s0)
                        ck_lo = s0 * SUB
                        ck_w = sum(sub_geom[i0 + s0 + j][3] for j in range(ns))
                        ps = psum_pool.tile([128, 4 * SUB], f32, tag="ps", name="ps")
                        for (jj, s, wd, st, sp, i, klo) in wjobs:
                            if not (s0 <= s < s0 + ns):
                                continue
                            sl = (jj - j0) * SW
                            nc.tensor.matmul(
                                out=ps[:, (s - s0) * SUB:(s - s0) * SUB + wd],
                                lhsT=oh_t[oo:oo + KROW, sl:sl + 128],
                                rhs=oh_t[oo:oo + KROW, sl + 128:sl + 128 + wd],
                                start=st, stop=sp)
                        nc.vector.scalar_tensor_tensor(
                            out=t[:, ck_lo:ck_lo + ck_w],
                            in0=t[:, ck_lo:ck_lo + ck_w],
                            scalar=p_t[g][:, :1],
                            in1=ps[:, :ck_w],
                            op0=mybir.AluOpType.mult,
                            op1=mybir.AluOpType.add)
                    nc.scalar.dma_start(out=out[g][:, c0w:c0w + ww], in_=t[:, :ww])
    nc.finalize()
    return nc


def kernel(vocab_ds, attns, p_gens, sources, decoder_batch_len):
    T = int(decoder_batch_len)
    in_maps, meta = _host_prep(vocab_ds, attns, p_gens, sources, T)
    nc = _build_nc(meta)

    from concourse.bass_utils import run_bass_kernel_spmd
    res = run_bass_kernel_spmd(nc, in_maps, list(range(N_CORES)))

    rows, V = np.asarray(vocab_ds).shape
    B, BPC, G = meta["B"], meta["BPC"], meta["G"]
    full = np.empty((rows, V), dtype=np.float32)
    for core in range(N_CORES):
        for g in range(G):
            blk = np.asarray(res.results[core][f"out{g}"], dtype=np.float32)
            for half in range(2):
                b = core * BPC + g * 2 + half
                full[b::B] = blk[half * T:(half + 1) * T]
    return full


# revision 12
# speedup vs baseline: 1.5414x; 1.0363x over previous
"""Trainium2 Bass kernel for pointer-generator final-distribution (scatter_memory).

out[r, v] = p_gens[r] * vocab_ds[r, v]  (+ (1-p_gens[r])*attns[r, l_win]  at
v == sources[l, b(r)], duplicate source ids resolved last-occurrence-wins)

Strategy (8 NeuronCores, SPMD), bf16 streaming (DMA/HBM-bound):
  - Pure memory streaming: every element is read once and written once.
    Per-core HBM limit is ~358 GB/s, so bytes are the only lever. rel-err
    tolerance is 2e-2; bf16 (~1% err) halves traffic vs f32: host uploads
    vocab as bf16, device computes/stores bf16, host upconverts.
  - Shard by batch column: core k owns b in {4k..4k+3}, all T decoder steps
    (rows r = t*B + b). Host pre-gathers rows b-major so device DMAs are
    contiguous; two 128-row groups per core (2 b's x 64 t each).
  - One engine, one job (no cross-engine semaphore stalls in a stream):
    SP(sync) issues all loads, ACT does the per-partition p-scale,
    PE matmuls host-baked one-hots into PSUM (scatter projection),
    DVE merges PSUM into the tile, GPSIMD issues stores via SWDGE.
  - Scatter: for each 1024-wide subtile, host bakes [K, 128] bf16 update
    values (block-diagonal over the two b's) and the one-hot [K, 1024]
    selector (is_equal was measured to cost 92us of DVE - host-baking it
    is ~6 MB of extra DMA instead). PE computes vals.T @ onehot -> PSUM;
    DVE adds 2048-wide PSUM chunks into the streamed tile (PSUM operands
    run DVE at 1x, so coalescing to 2048 amortizes instr overhead).
"""

import numpy as np

N_CORES = 8
WIN = 8192
SUB = 512
KROW = 16


def _host_prep(vocab_ds, attns, p_gens, sources, T):
    import ml_dtypes
    f32 = np.float32
    bf16 = ml_dtypes.bfloat16
    vocab_ds = np.ascontiguousarray(vocab_ds, dtype=f32)
    attns = np.ascontiguousarray(attns, dtype=f32)
    p_gens = np.ascontiguousarray(p_gens, dtype=f32)
    src = np.asarray(sources).astype(np.int64)
    rows, V = vocab_ds.shape
    L, B = src.shape
    assert rows == T * B

    ag = (f32(1.0) - p_gens) * attns  # gated copy dist, [rows, L]

    # winners per batch column: duplicate source ids -> last occurrence wins
    wins = []
    for b in range(B):
        d = {}
        col = src[:, b]
        for l in range(L):
            d[int(col[l])] = l
        cols = np.fromiter(d.keys(), dtype=np.int64)
        ls = np.fromiter(d.values(), dtype=np.int64)
        o = np.argsort(cols)
        wins.append((cols[o], ls[o]))

    # global 512-col subtile grid
    NS = (V + SUB - 1) // SUB
    sub_geom = [(i * SUB, min(SUB, V - i * SUB)) for i in range(NS)]  # (c0, wd)

    # per-group window tables; the tail of the LAST-processed group is cut
    # into 2048-col units so the pipeline drains in small steps
    def windows_for(split_tail):
        wt = []
        c = 0
        while c < V:
            if split_tail and V - c <= 10240 + WIN - 8192 and V - c > WIN // 4:
                ww = min(WIN // 4, V - c)
            else:
                ww = min(WIN, V - c)
            if V - c - ww < SUB and V - c - ww > 0:
                ww = V - c  # never leave a sub-SUB sliver
            wt.append((c, ww))
            c += ww
        return wt

    BPC = B // N_CORES  # 4
    G = BPC // 2        # 2 groups of 2 b's
    wins_tbl = [windows_for(g == G - 1) for g in range(G)]

    # bucket updates per (core, g, subtile)
    upd = [[[[] for _ in range(NS)] for _ in range(G)] for _ in range(N_CORES)]
    for core in range(N_CORES):
        for g in range(G):
            for half in range(2):
                b = core * BPC + g * 2 + half
                cols, ls = wins[b]
                for c, l in zip(cols.tolist(), ls.tolist()):
                    upd[core][g][c // SUB].append((half, c, l))

    # uniform-per-(g, subtile) K across cores; >=1 so every PSUM region is
    # written (a zero one-hot row yields zeros after start=True reset)
    K_ws = [[max(1, max(len(upd[core][g][i]) for core in range(N_CORES)))
             for i in range(NS)] for g in range(G)]
    assert all(k <= 2 * KROW for g in range(G) for k in K_ws[g]), \
        "subtile update count exceeds 2*KROW"
    # Every scatter job is a [KROW, 128+SUB] block (vals | one-hot); rows
    # beyond the real K are zero (numerically exact). Subtiles with
    # K > KROW get a second accumulate job (start=False). Exactly KROW
    # rows per DMA keeps the row->SDMA-engine round-robin balanced.
    # jobs[g][w] = list of (jidx, s_local, wd, start, stop, i, klo)
    jobs = [[[] for _ in range(len(wins_tbl[g]))] for g in range(G)]
    win_info = []  # per (g,w): (i0, nsub, j0, njobs)
    NJ = []
    for g in range(G):
        wi = []
        jidx = 0
        for w, (c0w, ww) in enumerate(wins_tbl[g]):
            i0 = c0w // SUB
            nsub = (ww + SUB - 1) // SUB
            j0 = jidx
            for s in range(nsub):
                i = i0 + s
                K = K_ws[g][i]
                wd = sub_geom[i][1]
                if K <= KROW:
                    jobs[g][w].append((jidx, s, wd, True, True, i, 0))
                    jidx += 1
                else:
                    jobs[g][w].append((jidx, s, wd, True, False, i, 0))
                    jobs[g][w].append((jidx + 1, s, wd, False, True, i, KROW))
                    jidx += 2
            wi.append((i0, nsub, j0, jidx - j0))
        win_info.append(wi)
        NJ.append(jidx)

    # per-core device inputs
    in_maps = []
    for core in range(N_CORES):
        m = {}
        for g in range(G):
            row_idx = []
            for half in range(2):
                b = core * BPC + g * 2 + half
                row_idx.extend(t * B + b for t in range(T))
            row_idx = np.asarray(row_idx)
            m[f"vocab{g}"] = vocab_ds[row_idx].astype(bf16)
            m[f"pgen{g}"] = p_gens[row_idx]
            # merged per-job [KROW, 128 vals | 512 one-hot] blocks so the
            # scatter operands ship as one balanced DMA stream
            W = 128 + SUB
            ohv = np.zeros((KROW, NJ[g] * W), dtype=f32)
            for w in range(len(wins_tbl[g])):
                for (jj, s, wd, st, sp, i, klo) in jobs[g][w]:
                    c0 = sub_geom[i][0]
                    ups = upd[core][g][i][klo:klo + KROW]
                    for k, (half, c, l) in enumerate(ups):
                        r0 = half * T
                        ohv[k, jj * W + r0: jj * W + r0 + T] = \
                            ag[row_idx[r0: r0 + T], l]
                        ohv[k, jj * W + 128 + (c - c0)] = 1.0
            m[f"ohv{g}"] = ohv.astype(bf16)
        in_maps.append(m)

    meta = dict(V=V, T=T, B=B, NS=NS, G=G, sub_geom=sub_geom,
                K_ws=K_ws, BPC=BPC, win_info=win_info,
                jobs=jobs, NJ=NJ, wins_tbl=wins_tbl)
    return in_maps, meta


def _build_nc(meta):
    from concourse import bacc, mybir

    V, NS, G = meta["V"], meta["NS"], meta["G"]
    sub_geom, K_ws = meta["sub_geom"], meta["K_ws"]
    wins_tbl = meta["wins_tbl"]
    f32 = mybir.dt.float32

    bf16 = mybir.dt.bfloat16
    nc = bacc.Bacc(None, target_bir_lowering=False, debug=False)
    vocab = [nc.declare_dram_parameter(f"vocab{g}", [128, V], bf16, isOutput=False)
             for g in range(G)]
    pgen = [nc.declare_dram_parameter(f"pgen{g}", [128, 1], f32, isOutput=False)
            for g in range(G)]
    SW = 128 + SUB
    NJ = meta["NJ"]
    ohv = [nc.declare_dram_parameter(f"ohv{g}", [KROW, NJ[g] * SW], bf16, isOutput=False)
           for g in range(G)]
    out = [nc.declare_dram_parameter(f"out{g}", [128, V], bf16, isOutput=True)
           for g in range(G)]

    from concourse.tile import TileContext

    win_info = meta["win_info"]
    jobs = meta["jobs"]
    # SBUF base partition alternates {0,64} per window so the narrow
    # (KROW-partition) ohv DMAs spread over both SDMA engine halves
    # (matmul requires base partition in {0,32,64}, lhsT/rhs bases equal)
    SW = 128 + SUB
    with TileContext(nc) as tc:
        with tc.tile_pool(name="io", bufs=7) as io_pool, \
             tc.tile_pool(name="small", bufs=1) as small, \
             tc.tile_pool(name="oh", bufs=3) as oh_pool, \
             tc.tile_pool(name="psum", bufs=2, space="PSUM") as psum_pool:

            p_t = []
            for g in range(G):
                p_t.append(small.tile([128, 1], f32, tag=f"p{g}", name=f"p{g}"))

            for g in range(G):
                for w, (c0w, ww) in enumerate(wins_tbl[g]):
                    i0, nsub, j0, njobs = win_info[g][w]
                    oo = 64 * ((w + g) % 2)
                    t = io_pool.tile([128, WIN], bf16, tag="io")
                    nc.sync.dma_start(out=t[:, :ww], in_=vocab[g][:, c0w:c0w + ww])
                    oh_t = oh_pool.tile([128, 18 * SW], bf16,
                                        tag="oh", name="oht")
                    nc.sync.dma_start(
                        out=oh_t[oo:oo + KROW, :njobs * SW],
                        in_=ohv[g][:, j0 * SW:(j0 + njobs) * SW])
                    if w == 0:
                        nc.sync.dma_start(out=p_t[g][:], in_=pgen[g][:])
                    # PSUM chunks of 4 subtiles (2048 cols); one fused DVE
                    # pass per chunk does scale+merge: t = t*p + ps
                    wjobs = jobs[g][w]
                    for s0 in range(0, nsub, 4):
                        ns = min(4, nsub - s0)
                        ck_lo = s0 * SUB
                        ck_w = sum(sub_geom[i0 + s0 + j][1] for j in range(ns))
                        ps = psum_pool.tile([128, 4 * SUB], f32, tag="ps", name="ps")
                        for (jj, s, wd, st, sp, i, klo) in wjobs:
                            if not (s0 <= s < s0 + ns):
                                continue
                            sl = (jj - j0) * SW
                            nc.tensor.matmul(
                                out=ps[:, (s - s0) * SUB:(s - s0) * SUB + wd],
                                lhsT=oh_t[oo:oo + KROW, sl:sl + 128],
                                rhs=oh_t[oo:oo + KROW, sl + 128:sl + 128 + wd],
                                start=st, stop=sp)
                        nc.vector.scalar_tensor_tensor(
                            out=t[:, ck_lo:ck_lo + ck_w],
                            in0=t[:, ck_lo:ck_lo + ck_w],
                            scalar=p_t[g][:, :1],
                            in1=ps[:, :ck_w],
                            op0=mybir.AluOpType.mult,
                            op1=mybir.AluOpType.add)
                    nc.scalar.dma_start(out=out[g][:, c0w:c0w + ww], in_=t[:, :ww])
    nc.finalize()
    return nc


def kernel(vocab_ds, attns, p_gens, sources, decoder_batch_len):
    T = int(decoder_batch_len)
    in_maps, meta = _host_prep(vocab_ds, attns, p_gens, sources, T)
    nc = _build_nc(meta)

    from concourse.bass_utils import run_bass_kernel_spmd
    res = run_bass_kernel_spmd(nc, in_maps, list(range(N_CORES)))

    rows, V = np.asarray(vocab_ds).shape
    B, BPC, G = meta["B"], meta["BPC"], meta["G"]
    full = np.empty((rows, V), dtype=np.float32)
    for core in range(N_CORES):
        for g in range(G):
            blk = np.asarray(res.results[core][f"out{g}"], dtype=np.float32)
            for half in range(2):
                b = core * BPC + g * 2 + half
                full[b::B] = blk[half * T:(half + 1) * T]
    return full
